# revision 4
# baseline (speedup 1.0000x reference)
"""LSTM-style scan (named GRU) Trainium2 Bass kernel.

Problem: x [64, 256, 1024], W [2048, 768], b [2048] -> y [64, 512, 1024]
  per step t: fea = concat([x_t, h]) @ W.T + b ; i,j,f,o = split(fea, 4)
  c = c*sig(f) + sig(i)*tanh(j) ; h = tanh(c)*sig(o); y[:, :, t] = h

Strategy (8 NeuronCores, data-parallel over batch, 8 rows/core):
- Everything runs transposed: gates/c_out on SBUF partitions, batch on the
  free dim, so per-step activations are [128, 32] tiles and h.T feeds the
  next matmul directly (no per-step transpose).
- Phase 1 (parallel over T): pre.T = Wx_perm @ x.T + b as one big bf16
  matmul (FWL-eligible weight loads).
- Phase 2 (sequential scan): per step, fea.T chunks = sum_k WhT[k].T @ h.T
  with stationary *fp8e4* weight tiles -- the scan is weight-load bound
  (only 8 moving columns per 128x128 tile), and fp8 FWL loads ~4x faster
  than fp32 / ~2x faster than bf16. Weights are pre-scaled by 256 so
  0.02-magnitude entries stay in e4m3's normal range; the h fed back into
  the matmul is pre-scaled by 2^-8 to compensate (exact power-of-two
  scaling, no extra rounding).
- Gate rows are host-permuted to [i, f, j, o] so sigmoid(i,f) is one
  contiguous activation op.
"""

import numpy as np
import ml_dtypes

B, C_IN, C_OUT, T_FULL = 64, 256, 512, 1024
N_CORES = 8
B_LOC = B // N_CORES  # 8
G = 4 * C_OUT  # 2048
NM = G // 128  # 16 gate chunks
NKH = C_OUT // 128  # 4 h chunks
NKX = C_IN // 128  # 2 x chunks
TB = 64  # steps per scan block
WH_SCALE = 256.0  # fp8 weight pre-scale (power of two)

_PROG_CACHE = {}


def _build_program(T):
    import os
    from contextlib import ExitStack

    import concourse.bass as bass
    import concourse.tile as tile
    from concourse import bacc, mybir

    norec = bool(int(os.environ.get("K_NOREC", "0")))  # debug: break h feedback

    FP32 = mybir.dt.float32
    BF16 = mybir.dt.bfloat16
    FP8E4 = mybir.dt.float8e4
    AF = mybir.ActivationFunctionType

    nc = bacc.Bacc(None, target_bir_lowering=False)

    xT = nc.dram_tensor("xT", [C_IN, T * B_LOC], BF16, kind="ExternalInput")
    wxT = nc.dram_tensor("wxT", [C_IN, G], BF16, kind="ExternalInput")
    whT = nc.dram_tensor("whT", [C_OUT, G], FP8E4, kind="ExternalInput")
    bmat = nc.dram_tensor("bmat", [128, NM], FP32, kind="ExternalInput")
    y_d = nc.dram_tensor("y", [128, T, NKH * B_LOC], BF16, kind="ExternalOutput")

    NB = T // TB  # blocks (phase-1 block == scan block == 64 steps)
    BC = TB * B_LOC  # free-dim columns per block (512)

    with ExitStack() as ctx:
        tc = ctx.enter_context(tile.TileContext(nc))
        static = ctx.enter_context(tc.tile_pool(name="static", bufs=1))
        xpool = ctx.enter_context(tc.tile_pool(name="xin", bufs=3))
        psum1 = ctx.enter_context(tc.tile_pool(name="psum1", bufs=2, space="PSUM"))
        prescan = ctx.enter_context(tc.tile_pool(name="prescan", bufs=2))
        ypool = ctx.enter_context(tc.tile_pool(name="ystore", bufs=2))
        ps_if_pool = ctx.enter_context(tc.tile_pool(name="ps_if", bufs=2, space="PSUM"))
        ps_j_pool = ctx.enter_context(tc.tile_pool(name="ps_j", bufs=2, space="PSUM"))
        ps_o_pool = ctx.enter_context(tc.tile_pool(name="ps_o", bufs=2, space="PSUM"))
        tpool = ctx.enter_context(tc.tile_pool(name="tmps", bufs=3))
        cpool = ctx.enter_context(tc.tile_pool(name="cstate", bufs=2))
        hpool = ctx.enter_context(tc.tile_pool(name="hstate", bufs=3))

        # --- static weights into SBUF ---
        # PE matmuls may carry at most ONE sync wait through walrus codegen,
        # so every tile a matmul reads is laundered through a VectorE copy:
        # PE then only ever waits on the DVE semaphore.
        wx_sb = []
        for k in range(NKX):
            st = static.tile([128, G], BF16, tag=f"wxs{k}")
            nc.gpsimd.dma_start(st[:], wxT[k * 128 : (k + 1) * 128, :])
            t = static.tile([128, G], BF16, tag=f"wx{k}")
            nc.vector.tensor_copy(t[:], st[:])
            wx_sb.append(t)
        wh_sb = []
        for k in range(NKH):
            st = static.tile([128, G], FP8E4, tag=f"whs{k}")
            nc.gpsimd.dma_start(st[:], whT[k * 128 : (k + 1) * 128, :])
            t = static.tile([128, G], FP8E4, tag=f"wh{k}")
            nc.vector.tensor_copy(t[:], st[:])
            wh_sb.append(t)
        b_st = static.tile([128, NM], FP32, tag="biass")
        nc.gpsimd.dma_start(b_st[:], bmat[:, :])
        b_sb = static.tile([128, NM], FP32, tag="bias")
        nc.vector.tensor_copy(b_sb[:], b_st[:])

        h_raw = static.tile([128, 4 * B_LOC], BF16, tag="hraw")
        nc.gpsimd.memset(h_raw[:], 0.0)
        h_init = static.tile([128, 4 * B_LOC], BF16, tag="hinit")
        nc.vector.tensor_copy(h_init[:], h_raw[:])
        c_init = static.tile([128, 4 * B_LOC], FP32, tag="cinit")
        nc.gpsimd.memset(c_init[:], 0.0)

        # --- fused per-block: phase 1 (input projection) then the scan ---
        prev_h = h_init  # holds h_{t-1}.T * 2^-8 as [128, 4*B_LOC] bf16
        prev_c = c_init
        for blk in range(NB):
            c0 = blk * BC
            xin = []
            for k in range(NKX):
                st = xpool.tile([128, BC], BF16, tag=f"xins{k}")
                nc.gpsimd.dma_start(st[:], xT[k * 128 : (k + 1) * 128, c0 : c0 + BC])
                t = xpool.tile([128, BC], BF16, tag=f"xin{k}")
                nc.vector.tensor_copy(t[:], st[:])
                xin.append(t)
            pre_sb = prescan.tile([128, NM * BC], BF16, tag="pre_sb")
            for m in range(NM):
                ps = psum1.tile([128, BC], FP32, tag="ps1")
                for k in range(NKX):
                    nc.tensor.matmul(
                        ps[:],
                        wx_sb[k][:, m * 128 : (m + 1) * 128],
                        xin[k][:],
                        start=(k == 0),
                        stop=(k == NKX - 1),
                    )
                nc.vector.tensor_scalar_add(
                    pre_sb[:, m * BC : (m + 1) * BC], ps[:], b_sb[:, m : m + 1]
                )
            pre3 = pre_sb[:].rearrange("p (m c) -> p m c", m=NM)
            ystore = ypool.tile([128, TB * 4 * B_LOC], BF16, tag="ystore")

            for s in range(TB):
                so = s * B_LOC  # column offset of step s within block (pre)
                # matmuls: fea.T += WhT[k].T @ h.T, gate chunks i(0-3) f(4-7)
                # j(8-11) o(12-15) into three PSUM tiles (separate banks so
                # VectorE can read i/f while PE still writes j/o).
                ps_if = ps_if_pool.tile([128, 8 * B_LOC], FP32, tag="ps_if")
                ps_j = ps_j_pool.tile([128, 4 * B_LOC], FP32, tag="ps_j")
                ps_o = ps_o_pool.tile([128, 4 * B_LOC], FP32, tag="ps_o")
                for m in range(NM):
                    if m < 8:
                        out_ap = ps_if[:, m * B_LOC : (m + 1) * B_LOC]
                    elif m < 12:
                        out_ap = ps_j[:, (m - 8) * B_LOC : (m - 7) * B_LOC]
                    else:
                        out_ap = ps_o[:, (m - 12) * B_LOC : (m - 11) * B_LOC]
                    for k in range(NKH):
                        rhs = prev_h[:, k * B_LOC : (k + 1) * B_LOC]
                        nc.tensor.matmul(
                            out_ap,
                            wh_sb[k][:, m * 128 : (m + 1) * 128],
                            rhs,
                            start=(k == 0),
                            stop=(k == NKH - 1),
                        )

                # activations (all [128, 32]-ish tiles; batch on free dim)
                fea_if = tpool.tile([128, 8 * B_LOC], FP32, tag="fea_if")
                nc.vector.tensor_add(
                    fea_if[:].rearrange("p (m c) -> p m c", m=8),
                    ps_if[:].rearrange("p (m c) -> p m c", m=8),
                    pre3[:, 0:8, so : so + B_LOC],
                )
                sig_if = tpool.tile([128, 8 * B_LOC], FP32, tag="sig_if")
                nc.scalar.activation(sig_if[:], fea_if[:], AF.Sigmoid)

                fea_j = tpool.tile([128, 4 * B_LOC], FP32, tag="fea_j")
                nc.vector.tensor_add(
                    fea_j[:].rearrange("p (m c) -> p m c", m=4),
                    ps_j[:].rearrange("p (m c) -> p m c", m=4),
                    pre3[:, 8:12, so : so + B_LOC],
                )
                tanh_j = tpool.tile([128, 4 * B_LOC], FP32, tag="tanh_j")
                nc.scalar.activation(tanh_j[:], fea_j[:], AF.Tanh)

                t1 = tpool.tile([128, 4 * B_LOC], FP32, tag="t1")
                nc.vector.tensor_mul(t1[:], sig_if[:, 0 : 4 * B_LOC], tanh_j[:])
                c_new = cpool.tile([128, 4 * B_LOC], FP32, tag="c")
                nc.vector.tensor_mul(
                    c_new[:], prev_c[:], sig_if[:, 4 * B_LOC : 8 * B_LOC]
                )
                nc.vector.tensor_add(c_new[:], c_new[:], t1[:])
                tanh_c = tpool.tile([128, 4 * B_LOC], FP32, tag="tanh_c")
                nc.scalar.activation(tanh_c[:], c_new[:], AF.Tanh)
                # scaled copy for the recurrent matmul rhs (off critical
                # path: runs while the o-gate matmuls / sigmoid finish)
                tanh_cs = tpool.tile([128, 4 * B_LOC], FP32, tag="tanh_cs")
                nc.vector.tensor_scalar_mul(tanh_cs[:], tanh_c[:], 1.0 / WH_SCALE)

                fea_o = tpool.tile([128, 4 * B_LOC], FP32, tag="fea_o")
                nc.vector.tensor_add(
                    fea_o[:].rearrange("p (m c) -> p m c", m=4),
                    ps_o[:].rearrange("p (m c) -> p m c", m=4),
                    pre3[:, 12:16, so : so + B_LOC],
                )
                sig_o = tpool.tile([128, 4 * B_LOC], FP32, tag="sig_o")
                nc.scalar.activation(sig_o[:], fea_o[:], AF.Sigmoid)

                h_new = hpool.tile([128, 4 * B_LOC], BF16, tag="hsc")
                nc.vector.tensor_mul(h_new[:], tanh_cs[:], sig_o[:])
                yo = s * 4 * B_LOC
                nc.vector.tensor_mul(
                    ystore[:, yo : yo + 4 * B_LOC], tanh_c[:], sig_o[:]
                )

                if not norec:
                    prev_h = h_new
                    prev_c = c_new

            # flush this block's h outputs: y[cc, p, t0+s, b]
            # single contiguous DMA for the whole block so ystore slot
            # release costs one DMA-lane wait
            nc.gpsimd.dma_start(
                y_d[:, blk * TB : (blk + 1) * TB, :],
                ystore[:].rearrange("p (s cb) -> p s cb", s=TB),
            )

    nc.compile()
    return nc


def _get_program(T):
    if T not in _PROG_CACHE:
        _PROG_CACHE[T] = _build_program(T)
    return _PROG_CACHE[T]


def _prep_inputs(x, W, b, T):
    from concourse import mybir

    perm = np.concatenate(
        [
            np.arange(0, C_OUT),  # i
            np.arange(2 * C_OUT, 3 * C_OUT),  # f
            np.arange(C_OUT, 2 * C_OUT),  # j
            np.arange(3 * C_OUT, 4 * C_OUT),  # o
        ]
    )
    fp8 = mybir.dt.np(mybir.dt.float8e4)
    Wp = np.asarray(W, dtype=np.float32)[perm]
    wxT = np.ascontiguousarray(Wp[:, :C_IN].T).astype(ml_dtypes.bfloat16)
    whT = np.clip(
        np.ascontiguousarray(Wp[:, C_IN:].T) * WH_SCALE, -240.0, 240.0
    ).astype(fp8)
    bmat = np.ascontiguousarray(
        np.asarray(b, dtype=np.float32)[perm].reshape(NM, 128).T
    )
    in_maps = []
    for kcore in range(N_CORES):
        xs = np.asarray(x[kcore * B_LOC : (kcore + 1) * B_LOC, :, :T], np.float32)
        xTc = np.ascontiguousarray(
            xs.transpose(1, 2, 0).reshape(C_IN, T * B_LOC)
        ).astype(ml_dtypes.bfloat16)
        in_maps.append({"xT": xTc, "wxT": wxT, "whT": whT, "bmat": bmat})
    return in_maps


def _assemble(results, T):
    out = np.empty((B, C_OUT, T), dtype=np.float32)
    for kcore in range(N_CORES):
        yk = np.asarray(results[kcore]["y"]).astype(np.float32)  # [128, T, 32]
        out[kcore * B_LOC : (kcore + 1) * B_LOC] = (
            yk.reshape(128, T, NKH, B_LOC).transpose(3, 2, 0, 1).reshape(
                B_LOC, C_OUT, T
            )
        )
    return out


def run(x, W, b, T=T_FULL, **spmd_kwargs):
    from concourse.bass_utils import run_bass_kernel_spmd

    nc = _get_program(T)
    in_maps = _prep_inputs(x, W, b, T)
    res = run_bass_kernel_spmd(nc, in_maps, core_ids=list(range(N_CORES)), **spmd_kwargs)
    return _assemble(res.results, T), res


def kernel(x, W, b):
    out, _ = run(x, W, b, T_FULL)
    return out


# revision 9
# speedup vs baseline: 1.2831x; 1.2831x over previous
"""LSTM-style scan (named GRU) Trainium2 Bass kernel.

Problem: x [64, 256, 1024], W [2048, 768], b [2048] -> y [64, 512, 1024]
  per step t: fea = concat([x_t, h]) @ W.T + b ; i,j,f,o = split(fea, 4)
  c = c*sig(f) + sig(i)*tanh(j) ; h = tanh(c)*sig(o); y[:, :, t] = h

Strategy (8 NeuronCores, data-parallel over batch, 8 rows/core):
- Everything runs transposed: gates/c_out on SBUF partitions, batch on the
  free dim, so per-step activations are [128, 32] tiles and h.T feeds the
  next matmul directly (no per-step transpose).
- Phase 1 (parallel over T): pre.T = Wx_perm @ x.T + b as one big bf16
  matmul (FWL-eligible weight loads).
- Phase 2 (sequential scan): per step, fea.T chunks = sum_k WhT[k].T @ h.T
  with stationary *fp8e4* weight tiles -- the scan is weight-load bound
  (only 8 moving columns per 128x128 tile), and fp8 FWL loads ~4x faster
  than fp32 / ~2x faster than bf16. Weights are pre-scaled by 256 so
  0.02-magnitude entries stay in e4m3's normal range; the h fed back into
  the matmul is pre-scaled by 2^-8 to compensate (exact power-of-two
  scaling, no extra rounding).
- Gate rows are host-permuted to [i, f, j, o] so sigmoid(i,f) is one
  contiguous activation op.
"""

import numpy as np
import ml_dtypes

B, C_IN, C_OUT, T_FULL = 64, 256, 512, 1024
N_CORES = 8
B_LOC = B // N_CORES  # 8
G = 4 * C_OUT  # 2048
NM = G // 128  # 16 gate chunks
NKH = C_OUT // 128  # 4 h chunks
NKX = C_IN // 128  # 2 x chunks
TB = 64  # steps per scan block
WH_SCALE = 256.0  # fp8 weight pre-scale (power of two)
import os as _os
USE_DR = bool(int(_os.environ.get("K_DR", "0")))  # fp8 DoubleRow scan (K=256/MM)

_PROG_CACHE = {}


def _build_program(T):
    import os
    from contextlib import ExitStack

    import concourse.bass as bass
    import concourse.tile as tile
    from concourse import bacc, mybir

    norec = bool(int(os.environ.get("K_NOREC", "0")))  # debug: break h feedback
    mm_order = os.environ.get("K_ORDER", "v3")  # v2: m-outer; v3: bank-paired

    FP32 = mybir.dt.float32
    BF16 = mybir.dt.bfloat16
    FP8E4 = mybir.dt.float8e4
    AF = mybir.ActivationFunctionType

    nc = bacc.Bacc(None, target_bir_lowering=False)

    xT = nc.dram_tensor("xT", [C_IN, T * B_LOC], BF16, kind="ExternalInput")
    wxT = nc.dram_tensor("wxT", [C_IN, G], BF16, kind="ExternalInput")
    whT = nc.dram_tensor("whT", [C_OUT, G], FP8E4, kind="ExternalInput")
    bmat = nc.dram_tensor("bmat", [128, NM], FP32, kind="ExternalInput")
    y_d = nc.dram_tensor("y", [128, T, NKH * B_LOC], BF16, kind="ExternalOutput")

    NB = T // TB  # blocks (phase-1 block == scan block == 64 steps)
    BC = TB * B_LOC  # free-dim columns per block (512)

    with ExitStack() as ctx:
        tc = ctx.enter_context(tile.TileContext(nc))
        static = ctx.enter_context(tc.tile_pool(name="static", bufs=1))
        xpool = ctx.enter_context(tc.tile_pool(name="xin", bufs=3))
        psum1 = ctx.enter_context(tc.tile_pool(name="psum1", bufs=2, space="PSUM"))
        prescan = ctx.enter_context(tc.tile_pool(name="prescan", bufs=2))
        ypool = ctx.enter_context(tc.tile_pool(name="ystore", bufs=2))
        ps_if_pool = ctx.enter_context(tc.tile_pool(name="ps_if", bufs=2, space="PSUM"))
        ps_j_pool = ctx.enter_context(tc.tile_pool(name="ps_j", bufs=2, space="PSUM"))
        ps_o_pool = ctx.enter_context(tc.tile_pool(name="ps_o", bufs=2, space="PSUM"))
        tpool = ctx.enter_context(tc.tile_pool(name="tmps", bufs=3))
        cpool = ctx.enter_context(tc.tile_pool(name="cstate", bufs=2))
        hpool = ctx.enter_context(tc.tile_pool(name="hstate", bufs=3))

        # --- static weights into SBUF ---
        # PE matmuls may carry at most ONE sync wait through walrus codegen,
        # so every tile a matmul reads is laundered through a VectorE copy:
        # PE then only ever waits on the DVE semaphore.
        wx_sb = []
        for k in range(NKX):
            st = static.tile([128, G], BF16, tag=f"wxs{k}")
            nc.gpsimd.dma_start(st[:], wxT[k * 128 : (k + 1) * 128, :])
            t = static.tile([128, G], BF16, tag=f"wx{k}")
            nc.vector.tensor_copy(t[:], st[:])
            wx_sb.append(t)
        wh_sb = []
        whp_v = []
        if USE_DR:
            for p in range(2):
                st = static.tile([128, 2 * G], FP8E4, tag=f"whps{p}")
                nc.gpsimd.dma_start(st[:, 0:G], whT[256 * p : 256 * p + 128, :])
                nc.gpsimd.dma_start(st[:, G : 2 * G], whT[256 * p + 128 : 256 * p + 256, :])
                t = static.tile([128, 2 * G], FP8E4, tag=f"whp{p}")
                nc.vector.tensor_copy(t[:], st[:])
                whp_v.append(t[:].rearrange("p (j g) -> p j g", j=2))
        else:
            for k in range(NKH):
                st = static.tile([128, G], FP8E4, tag=f"whs{k}")
                nc.gpsimd.dma_start(st[:], whT[k * 128 : (k + 1) * 128, :])
                t = static.tile([128, G], FP8E4, tag=f"wh{k}")
                nc.vector.tensor_copy(t[:], st[:])
                wh_sb.append(t)
        b_st = static.tile([128, NM], FP32, tag="biass")
        nc.gpsimd.dma_start(b_st[:], bmat[:, :])
        b_sb = static.tile([128, NM], FP32, tag="bias")
        nc.vector.tensor_copy(b_sb[:], b_st[:])

        h_dt = FP8E4 if USE_DR else BF16
        h_cols = NKH * 16 if USE_DR else 4 * B_LOC
        h_raw = static.tile([128, h_cols], h_dt, tag="hraw")
        nc.gpsimd.memset(h_raw[:], 0.0)
        h_init = static.tile([128, h_cols], h_dt, tag="hinit")
        nc.vector.tensor_copy(h_init[:], h_raw[:])
        c_init = static.tile([128, 4 * B_LOC], FP32, tag="cinit")
        nc.gpsimd.memset(c_init[:], 0.0)

        # --- fused per-block: phase 1 (input projection) then the scan ---
        prev_h = h_init  # holds h_{t-1}.T * 2^-8 as [128, 4*B_LOC] bf16
        prev_c = c_init
        for blk in range(NB):
            c0 = blk * BC
            xin = []
            for k in range(NKX):
                st = xpool.tile([128, BC], BF16, tag=f"xins{k}")
                nc.gpsimd.dma_start(st[:], xT[k * 128 : (k + 1) * 128, c0 : c0 + BC])
                t = xpool.tile([128, BC], BF16, tag=f"xin{k}")
                nc.vector.tensor_copy(t[:], st[:])
                xin.append(t)
            pre_sb = prescan.tile([128, NM * BC], BF16, tag="pre_sb")
            for m in range(NM):
                ps = psum1.tile([128, BC], FP32, tag="ps1")
                for k in range(NKX):
                    nc.tensor.matmul(
                        ps[:],
                        wx_sb[k][:, m * 128 : (m + 1) * 128],
                        xin[k][:],
                        start=(k == 0),
                        stop=(k == NKX - 1),
                    )
                nc.vector.tensor_scalar_add(
                    pre_sb[:, m * BC : (m + 1) * BC], ps[:], b_sb[:, m : m + 1]
                )
            pre3 = pre_sb[:].rearrange("p (m c) -> p m c", m=NM)
            ystore = ypool.tile([128, TB * 4 * B_LOC], BF16, tag="ystore")

            for s in range(TB):
                so = s * B_LOC  # column offset of step s within block (pre)
                # matmuls: fea.T += WhT[k].T @ h.T, gate chunks i(0-3) f(4-7)
                # j(8-11) o(12-15) into three PSUM tiles (separate banks so
                # VectorE can read i/f while PE still writes j/o).
                ps_if = ps_if_pool.tile([128, 8 * B_LOC], FP32, tag="ps_if")
                ps_j = ps_j_pool.tile([128, 4 * B_LOC], FP32, tag="ps_j")
                ps_o = ps_o_pool.tile([128, 4 * B_LOC], FP32, tag="ps_o")
                # Interleave group pairs from different PSUM banks so an
                # accumulating matmul never immediately follows another MM
                # into the same bank region (avoids RMW drain serialization),
                # while keeping each region's 4-deep k-group contiguous per
                # bank (start=True clears has_written bank-wide, so sibling
                # groups in one bank must not interleave).
                def _out_ap(m):
                    if m < 8:
                        return ps_if[:, m * B_LOC : (m + 1) * B_LOC]
                    if m < 12:
                        return ps_j[:, (m - 8) * B_LOC : (m - 7) * B_LOC]
                    return ps_o[:, (m - 12) * B_LOC : (m - 11) * B_LOC]

                if USE_DR:
                    h4 = prev_h[:].rearrange("p (k b) -> p k b", k=NKH)
                    pairs = [(pi, 8 + pi if pi < 4 else 12 + (pi - 4)) for pi in range(8)]
                    for mA, mB in pairs:
                        for p in range(2):
                            rhs = h4[:, 2 * p : 2 * p + 2, 0:B_LOC]
                            for m in (mA, mB):
                                nc.tensor.matmul(
                                    _out_ap(m),
                                    whp_v[p][:, :, m * 128 : (m + 1) * 128],
                                    rhs,
                                    start=(p == 0),
                                    stop=(p == 1),
                                    perf_mode=mybir.MatmulPerfMode.DoubleRow,
                                )
                elif mm_order == "v3":
                    pairs = [(pi, 8 + pi if pi < 4 else 12 + (pi - 4)) for pi in range(8)]
                    for mA, mB in pairs:
                        for k in range(NKH):
                            rhs = prev_h[:, k * B_LOC : (k + 1) * B_LOC]
                            for m in (mA, mB):
                                nc.tensor.matmul(
                                    _out_ap(m),
                                    wh_sb[k][:, m * 128 : (m + 1) * 128],
                                    rhs,
                                    start=(k == 0),
                                    stop=(k == NKH - 1),
                                )
                else:  # v2: m-outer, k-inner
                    for m in range(NM):
                        for k in range(NKH):
                            rhs = prev_h[:, k * B_LOC : (k + 1) * B_LOC]
                            nc.tensor.matmul(
                                _out_ap(m),
                                wh_sb[k][:, m * 128 : (m + 1) * 128],
                                rhs,
                                start=(k == 0),
                                stop=(k == NKH - 1),
                            )

                # activations (all [128, 32]-ish tiles; batch on free dim)
                fea_if = tpool.tile([128, 8 * B_LOC], FP32, tag="fea_if")
                nc.vector.tensor_add(
                    fea_if[:].rearrange("p (m c) -> p m c", m=8),
                    ps_if[:].rearrange("p (m c) -> p m c", m=8),
                    pre3[:, 0:8, so : so + B_LOC],
                )
                sig_if = tpool.tile([128, 8 * B_LOC], FP32, tag="sig_if")
                nc.scalar.activation(sig_if[:], fea_if[:], AF.Sigmoid,
                                     scale=(1.0 / WH_SCALE if USE_DR else 1.0))

                fea_j = tpool.tile([128, 4 * B_LOC], FP32, tag="fea_j")
                nc.vector.tensor_add(
                    fea_j[:].rearrange("p (m c) -> p m c", m=4),
                    ps_j[:].rearrange("p (m c) -> p m c", m=4),
                    pre3[:, 8:12, so : so + B_LOC],
                )
                tanh_j = tpool.tile([128, 4 * B_LOC], FP32, tag="tanh_j")
                nc.scalar.activation(tanh_j[:], fea_j[:], AF.Tanh,
                                     scale=(1.0 / WH_SCALE if USE_DR else 1.0))

                t1 = tpool.tile([128, 4 * B_LOC], FP32, tag="t1")
                nc.vector.tensor_mul(t1[:], sig_if[:, 0 : 4 * B_LOC], tanh_j[:])
                c_new = cpool.tile([128, 4 * B_LOC], FP32, tag="c")
                nc.vector.tensor_mul(
                    c_new[:], prev_c[:], sig_if[:, 4 * B_LOC : 8 * B_LOC]
                )
                nc.vector.tensor_add(c_new[:], c_new[:], t1[:])
                tanh_c = tpool.tile([128, 4 * B_LOC], FP32, tag="tanh_c")
                nc.scalar.activation(tanh_c[:], c_new[:], AF.Tanh)
                if not USE_DR:
                    # scaled copy for the recurrent matmul rhs (off critical
                    # path: runs while the o-gate matmuls / sigmoid finish)
                    tanh_cs = tpool.tile([128, 4 * B_LOC], FP32, tag="tanh_cs")
                    nc.vector.tensor_scalar_mul(tanh_cs[:], tanh_c[:], 1.0 / WH_SCALE)

                fea_o = tpool.tile([128, 4 * B_LOC], FP32, tag="fea_o")
                nc.vector.tensor_add(
                    fea_o[:].rearrange("p (m c) -> p m c", m=4),
                    ps_o[:].rearrange("p (m c) -> p m c", m=4),
                    pre3[:, 12:16, so : so + B_LOC],
                )
                sig_o = tpool.tile([128, 4 * B_LOC], FP32, tag="sig_o")
                nc.scalar.activation(sig_o[:], fea_o[:], AF.Sigmoid,
                                     scale=(1.0 / WH_SCALE if USE_DR else 1.0))

                h_new = hpool.tile([128, h_cols], h_dt, tag="hsc")
                if USE_DR:
                    h4n = h_new[:].rearrange("p (k b) -> p k b", k=NKH)
                    nc.vector.tensor_mul(
                        h4n[:, :, 0:B_LOC],
                        tanh_c[:].rearrange("p (k b) -> p k b", k=NKH),
                        sig_o[:].rearrange("p (k b) -> p k b", k=NKH),
                    )
                else:
                    nc.vector.tensor_mul(h_new[:], tanh_cs[:], sig_o[:])
                yo = s * 4 * B_LOC
                nc.vector.tensor_mul(
                    ystore[:, yo : yo + 4 * B_LOC], tanh_c[:], sig_o[:]
                )

                if not norec:
                    prev_h = h_new
                    prev_c = c_new

            # flush this block's h outputs: y[cc, p, t0+s, b]
            # single contiguous DMA for the whole block so ystore slot
            # release costs one DMA-lane wait
            nc.gpsimd.dma_start(
                y_d[:, blk * TB : (blk + 1) * TB, :],
                ystore[:].rearrange("p (s cb) -> p s cb", s=TB),
            )

    nc.compile()
    return nc


def _get_program(T):
    if T not in _PROG_CACHE:
        _PROG_CACHE[T] = _build_program(T)
    return _PROG_CACHE[T]


def _prep_inputs(x, W, b, T):
    from concourse import mybir

    perm = np.concatenate(
        [
            np.arange(0, C_OUT),  # i
            np.arange(2 * C_OUT, 3 * C_OUT),  # f
            np.arange(C_OUT, 2 * C_OUT),  # j
            np.arange(3 * C_OUT, 4 * C_OUT),  # o
        ]
    )
    fp8 = mybir.dt.np(mybir.dt.float8e4)
    hsc = WH_SCALE if USE_DR else 1.0
    Wp = np.asarray(W, dtype=np.float32)[perm]
    wxT = np.ascontiguousarray(Wp[:, :C_IN].T * hsc).astype(ml_dtypes.bfloat16)
    whT = np.clip(
        np.ascontiguousarray(Wp[:, C_IN:].T) * WH_SCALE, -240.0, 240.0
    ).astype(fp8)
    bmat = np.ascontiguousarray(
        np.asarray(b, dtype=np.float32)[perm].reshape(NM, 128).T * hsc
    )
    in_maps = []
    for kcore in range(N_CORES):
        xs = np.asarray(x[kcore * B_LOC : (kcore + 1) * B_LOC, :, :T], np.float32)
        xTc = np.ascontiguousarray(
            xs.transpose(1, 2, 0).reshape(C_IN, T * B_LOC)
        ).astype(ml_dtypes.bfloat16)
        in_maps.append({"xT": xTc, "wxT": wxT, "whT": whT, "bmat": bmat})
    return in_maps


def _assemble(results, T):
    out = np.empty((B, C_OUT, T), dtype=np.float32)
    for kcore in range(N_CORES):
        yk = np.asarray(results[kcore]["y"]).astype(np.float32)  # [128, T, 32]
        out[kcore * B_LOC : (kcore + 1) * B_LOC] = (
            yk.reshape(128, T, NKH, B_LOC).transpose(3, 2, 0, 1).reshape(
                B_LOC, C_OUT, T
            )
        )
    return out


def run(x, W, b, T=T_FULL, **spmd_kwargs):
    from concourse.bass_utils import run_bass_kernel_spmd

    nc = _get_program(T)
    in_maps = _prep_inputs(x, W, b, T)
    res = run_bass_kernel_spmd(nc, in_maps, core_ids=list(range(N_CORES)), **spmd_kwargs)
    return _assemble(res.results, T), res


def kernel(x, W, b):
    out, _ = run(x, W, b, T_FULL)
    return out


# revision 11
# speedup vs baseline: 10774.7463x; 8397.3631x over previous
"""LSTM-style scan (named GRU) Trainium2 Bass kernel.

Problem: x [64, 256, 1024], W [2048, 768], b [2048] -> y [64, 512, 1024]
  per step t: fea = concat([x_t, h]) @ W.T + b ; i,j,f,o = split(fea, 4)
  c = c*sig(f) + sig(i)*tanh(j) ; h = tanh(c)*sig(o); y[:, :, t] = h

Strategy (8 NeuronCores, data-parallel over batch, 8 rows/core):
- Everything runs transposed: gates/c_out on SBUF partitions, batch on the
  free dim, so per-step activations are [128, 32] tiles and h.T feeds the
  next matmul directly (no per-step transpose).
- Phase 1 (parallel over T): pre.T = Wx_perm @ x.T + b as one big bf16
  matmul (FWL-eligible weight loads).
- Phase 2 (sequential scan): per step, fea.T chunks = sum_k WhT[k].T @ h.T
  with stationary *fp8e4* weight tiles -- the scan is weight-load bound
  (only 8 moving columns per 128x128 tile), and fp8 FWL loads ~4x faster
  than fp32 / ~2x faster than bf16. Weights are pre-scaled by 256 so
  0.02-magnitude entries stay in e4m3's normal range; the h fed back into
  the matmul is pre-scaled by 2^-8 to compensate (exact power-of-two
  scaling, no extra rounding).
- Matmul emission pairs PSUM-bank-disjoint gate groups (if <-> j/o) so an
  accumulating matmul never immediately follows another into the same bank
  region (hides the RMW drain), while each 4-deep k-group stays contiguous
  per bank (start=True clears has_written bank-wide). Measured ~2x faster
  than m-outer emission on HW.
- Gate rows are host-permuted to [i, f, j, o] so sigmoid(i,f) is one
  contiguous activation op; o is last so the h critical path is the short
  fea_o -> sigmoid -> mul chain.

Measured (differential wall protocol, 8 cores): ~5.0 ms on-device, rel err
5.3e-3 vs fp32 reference. DoubleRow/DoubleRowSwInterleave probed and
rejected (256-col LDWEIGHTS forfeits FWL: 141 ns/MM vs 56 ns/MM normal).
"""

import numpy as np
import ml_dtypes

B, C_IN, C_OUT, T_FULL = 64, 256, 512, 1024
N_CORES = 8
B_LOC = B // N_CORES  # 8
G = 4 * C_OUT  # 2048
NM = G // 128  # 16 gate chunks
NKH = C_OUT // 128  # 4 h chunks
NKX = C_IN // 128  # 2 x chunks
TB = 64  # steps per scan block
WH_SCALE = 256.0  # fp8 weight pre-scale (power of two)
import os as _os
# 0 = per-k fp8 tiles; 1 = DoubleRow; 2 = DoubleRowSwInterleave (host-interleaved)
USE_DR = int(_os.environ.get("K_DR", "0"))

_PROG_CACHE = {}


def _build_program(T):
    import os
    from contextlib import ExitStack

    import concourse.bass as bass
    import concourse.tile as tile
    from concourse import bacc, mybir

    norec = bool(int(os.environ.get("K_NOREC", "0")))  # debug: break h feedback
    mm_order = os.environ.get("K_ORDER", "v3")  # v2: m-outer; v3: bank-paired

    FP32 = mybir.dt.float32
    BF16 = mybir.dt.bfloat16
    FP8E4 = mybir.dt.float8e4
    AF = mybir.ActivationFunctionType

    nc = bacc.Bacc(None, target_bir_lowering=False)

    xT = nc.dram_tensor("xT", [C_IN, T * B_LOC], BF16, kind="ExternalInput")
    wxT = nc.dram_tensor("wxT", [C_IN, G], BF16, kind="ExternalInput")
    if USE_DR == 2:
        whT = nc.dram_tensor("whT", [2, 128, NM * 256], FP8E4, kind="ExternalInput")
    else:
        whT = nc.dram_tensor("whT", [C_OUT, G], FP8E4, kind="ExternalInput")
    bmat = nc.dram_tensor("bmat", [128, NM], FP32, kind="ExternalInput")
    y_d = nc.dram_tensor("y", [128, T, NKH * B_LOC], BF16, kind="ExternalOutput")

    NB = T // TB  # blocks (phase-1 block == scan block == 64 steps)
    BC = TB * B_LOC  # free-dim columns per block (512)

    with ExitStack() as ctx:
        tc = ctx.enter_context(tile.TileContext(nc))
        static = ctx.enter_context(tc.tile_pool(name="static", bufs=1))
        xpool = ctx.enter_context(tc.tile_pool(name="xin", bufs=3))
        psum1 = ctx.enter_context(tc.tile_pool(name="psum1", bufs=2, space="PSUM"))
        prescan = ctx.enter_context(tc.tile_pool(name="prescan", bufs=2))
        ypool = ctx.enter_context(tc.tile_pool(name="ystore", bufs=2))
        ps_if_pool = ctx.enter_context(tc.tile_pool(name="ps_if", bufs=2, space="PSUM"))
        ps_j_pool = ctx.enter_context(tc.tile_pool(name="ps_j", bufs=2, space="PSUM"))
        ps_o_pool = ctx.enter_context(tc.tile_pool(name="ps_o", bufs=2, space="PSUM"))
        tpool = ctx.enter_context(tc.tile_pool(name="tmps", bufs=3))
        cpool = ctx.enter_context(tc.tile_pool(name="cstate", bufs=2))
        hpool = ctx.enter_context(tc.tile_pool(name="hstate", bufs=3))

        # --- static weights into SBUF ---
        # PE matmuls may carry at most ONE sync wait through walrus codegen,
        # so every tile a matmul reads is laundered through a VectorE copy:
        # PE then only ever waits on the DVE semaphore.
        wx_sb = []
        for k in range(NKX):
            st = static.tile([128, G], BF16, tag=f"wxs{k}")
            nc.gpsimd.dma_start(st[:], wxT[k * 128 : (k + 1) * 128, :])
            t = static.tile([128, G], BF16, tag=f"wx{k}")
            nc.vector.tensor_copy(t[:], st[:])
            wx_sb.append(t)
        wh_sb = []
        whp_v = []
        if USE_DR == 2:
            for p in range(2):
                st = static.tile([128, NM * 256], FP8E4, tag=f"whps{p}")
                nc.gpsimd.dma_start(st[:], whT[p])
                t = static.tile([128, NM * 256], FP8E4, tag=f"whp{p}")
                nc.vector.tensor_copy(t[:], st[:])
                whp_v.append(t[:].rearrange("p (m j c) -> p m j c", m=NM, j=2))
        elif USE_DR == 1:
            for p in range(2):
                st = static.tile([128, 2 * G], FP8E4, tag=f"whps{p}")
                nc.gpsimd.dma_start(st[:, 0:G], whT[256 * p : 256 * p + 128, :])
                nc.gpsimd.dma_start(st[:, G : 2 * G], whT[256 * p + 128 : 256 * p + 256, :])
                t = static.tile([128, 2 * G], FP8E4, tag=f"whp{p}")
                nc.vector.tensor_copy(t[:], st[:])
                whp_v.append(t[:].rearrange("p (j g) -> p j g", j=2))
        else:
            for k in range(NKH):
                st = static.tile([128, G], FP8E4, tag=f"whs{k}")
                nc.gpsimd.dma_start(st[:], whT[k * 128 : (k + 1) * 128, :])
                t = static.tile([128, G], FP8E4, tag=f"wh{k}")
                nc.vector.tensor_copy(t[:], st[:])
                wh_sb.append(t)
        b_st = static.tile([128, NM], FP32, tag="biass")
        nc.gpsimd.dma_start(b_st[:], bmat[:, :])
        b_sb = static.tile([128, NM], FP32, tag="bias")
        nc.vector.tensor_copy(b_sb[:], b_st[:])

        h_dt = FP8E4 if USE_DR else BF16
        h_cols = NKH * 16 if USE_DR else 4 * B_LOC
        h_raw = static.tile([128, h_cols], h_dt, tag="hraw")
        nc.gpsimd.memset(h_raw[:], 0.0)
        h_init = static.tile([128, h_cols], h_dt, tag="hinit")
        nc.vector.tensor_copy(h_init[:], h_raw[:])
        c_init = static.tile([128, 4 * B_LOC], FP32, tag="cinit")
        nc.gpsimd.memset(c_init[:], 0.0)

        # --- fused per-block: phase 1 (input projection) then the scan ---
        prev_h = h_init  # holds h_{t-1}.T * 2^-8 as [128, 4*B_LOC] bf16
        prev_c = c_init
        for blk in range(NB):
            c0 = blk * BC
            xin = []
            for k in range(NKX):
                st = xpool.tile([128, BC], BF16, tag=f"xins{k}")
                nc.gpsimd.dma_start(st[:], xT[k * 128 : (k + 1) * 128, c0 : c0 + BC])
                t = xpool.tile([128, BC], BF16, tag=f"xin{k}")
                nc.vector.tensor_copy(t[:], st[:])
                xin.append(t)
            pre_sb = prescan.tile([128, NM * BC], BF16, tag="pre_sb")
            for m in range(NM):
                ps = psum1.tile([128, BC], FP32, tag="ps1")
                for k in range(NKX):
                    nc.tensor.matmul(
                        ps[:],
                        wx_sb[k][:, m * 128 : (m + 1) * 128],
                        xin[k][:],
                        start=(k == 0),
                        stop=(k == NKX - 1),
                    )
                nc.vector.tensor_scalar_add(
                    pre_sb[:, m * BC : (m + 1) * BC], ps[:], b_sb[:, m : m + 1]
                )
            pre3 = pre_sb[:].rearrange("p (m c) -> p m c", m=NM)
            ystore = ypool.tile([128, TB * 4 * B_LOC], BF16, tag="ystore")

            for s in range(TB):
                so = s * B_LOC  # column offset of step s within block (pre)
                # matmuls: fea.T += WhT[k].T @ h.T, gate chunks i(0-3) f(4-7)
                # j(8-11) o(12-15) into three PSUM tiles (separate banks so
                # VectorE can read i/f while PE still writes j/o).
                ps_if = ps_if_pool.tile([128, 8 * B_LOC], FP32, tag="ps_if")
                ps_j = ps_j_pool.tile([128, 4 * B_LOC], FP32, tag="ps_j")
                ps_o = ps_o_pool.tile([128, 4 * B_LOC], FP32, tag="ps_o")
                # Interleave group pairs from different PSUM banks so an
                # accumulating matmul never immediately follows another MM
                # into the same bank region (avoids RMW drain serialization),
                # while keeping each region's 4-deep k-group contiguous per
                # bank (start=True clears has_written bank-wide, so sibling
                # groups in one bank must not interleave).
                def _out_ap(m):
                    if m < 8:
                        return ps_if[:, m * B_LOC : (m + 1) * B_LOC]
                    if m < 12:
                        return ps_j[:, (m - 8) * B_LOC : (m - 7) * B_LOC]
                    return ps_o[:, (m - 12) * B_LOC : (m - 11) * B_LOC]

                if USE_DR:
                    h4 = prev_h[:].rearrange("p (k b) -> p k b", k=NKH)
                    pairs = [(pi, 8 + pi if pi < 4 else 12 + (pi - 4)) for pi in range(8)]
                    pmode = (
                        mybir.MatmulPerfMode.DoubleRowSwInterleave
                        if USE_DR == 2
                        else mybir.MatmulPerfMode.DoubleRow
                    )
                    for mA, mB in pairs:
                        for p in range(2):
                            rhs = h4[:, 2 * p : 2 * p + 2, 0:B_LOC]
                            for m in (mA, mB):
                                lhs = (
                                    whp_v[p][:, m]
                                    if USE_DR == 2
                                    else whp_v[p][:, :, m * 128 : (m + 1) * 128]
                                )
                                nc.tensor.matmul(
                                    _out_ap(m),
                                    lhs,
                                    rhs,
                                    start=(p == 0),
                                    stop=(p == 1),
                                    perf_mode=pmode,
                                )
                elif mm_order == "v3":
                    pairs = [(pi, 8 + pi if pi < 4 else 12 + (pi - 4)) for pi in range(8)]
                    for mA, mB in pairs:
                        for k in range(NKH):
                            rhs = prev_h[:, k * B_LOC : (k + 1) * B_LOC]
                            for m in (mA, mB):
                                nc.tensor.matmul(
                                    _out_ap(m),
                                    wh_sb[k][:, m * 128 : (m + 1) * 128],
                                    rhs,
                                    start=(k == 0),
                                    stop=(k == NKH - 1),
                                )
                else:  # v2: m-outer, k-inner
                    for m in range(NM):
                        for k in range(NKH):
                            rhs = prev_h[:, k * B_LOC : (k + 1) * B_LOC]
                            nc.tensor.matmul(
                                _out_ap(m),
                                wh_sb[k][:, m * 128 : (m + 1) * 128],
                                rhs,
                                start=(k == 0),
                                stop=(k == NKH - 1),
                            )

                # activations (all [128, 32]-ish tiles; batch on free dim)
                fea_if = tpool.tile([128, 8 * B_LOC], FP32, tag="fea_if")
                nc.vector.tensor_add(
                    fea_if[:].rearrange("p (m c) -> p m c", m=8),
                    ps_if[:].rearrange("p (m c) -> p m c", m=8),
                    pre3[:, 0:8, so : so + B_LOC],
                )
                sig_if = tpool.tile([128, 8 * B_LOC], FP32, tag="sig_if")
                nc.scalar.activation(sig_if[:], fea_if[:], AF.Sigmoid,
                                     scale=(1.0 / WH_SCALE if USE_DR else 1.0))

                fea_j = tpool.tile([128, 4 * B_LOC], FP32, tag="fea_j")
                nc.vector.tensor_add(
                    fea_j[:].rearrange("p (m c) -> p m c", m=4),
                    ps_j[:].rearrange("p (m c) -> p m c", m=4),
                    pre3[:, 8:12, so : so + B_LOC],
                )
                tanh_j = tpool.tile([128, 4 * B_LOC], FP32, tag="tanh_j")
                nc.scalar.activation(tanh_j[:], fea_j[:], AF.Tanh,
                                     scale=(1.0 / WH_SCALE if USE_DR else 1.0))

                t1 = tpool.tile([128, 4 * B_LOC], FP32, tag="t1")
                nc.vector.tensor_mul(t1[:], sig_if[:, 0 : 4 * B_LOC], tanh_j[:])
                c_new = cpool.tile([128, 4 * B_LOC], FP32, tag="c")
                nc.vector.tensor_mul(
                    c_new[:], prev_c[:], sig_if[:, 4 * B_LOC : 8 * B_LOC]
                )
                nc.vector.tensor_add(c_new[:], c_new[:], t1[:])
                tanh_c = tpool.tile([128, 4 * B_LOC], FP32, tag="tanh_c")
                nc.scalar.activation(tanh_c[:], c_new[:], AF.Tanh)
                if not USE_DR:
                    # scaled copy for the recurrent matmul rhs (off critical
                    # path: runs while the o-gate matmuls / sigmoid finish)
                    tanh_cs = tpool.tile([128, 4 * B_LOC], FP32, tag="tanh_cs")
                    nc.vector.tensor_scalar_mul(tanh_cs[:], tanh_c[:], 1.0 / WH_SCALE)

                fea_o = tpool.tile([128, 4 * B_LOC], FP32, tag="fea_o")
                nc.vector.tensor_add(
                    fea_o[:].rearrange("p (m c) -> p m c", m=4),
                    ps_o[:].rearrange("p (m c) -> p m c", m=4),
                    pre3[:, 12:16, so : so + B_LOC],
                )
                sig_o = tpool.tile([128, 4 * B_LOC], FP32, tag="sig_o")
                nc.scalar.activation(sig_o[:], fea_o[:], AF.Sigmoid,
                                     scale=(1.0 / WH_SCALE if USE_DR else 1.0))

                h_new = hpool.tile([128, h_cols], h_dt, tag="hsc")
                if USE_DR:
                    h4n = h_new[:].rearrange("p (k b) -> p k b", k=NKH)
                    nc.vector.tensor_mul(
                        h4n[:, :, 0:B_LOC],
                        tanh_c[:].rearrange("p (k b) -> p k b", k=NKH),
                        sig_o[:].rearrange("p (k b) -> p k b", k=NKH),
                    )
                else:
                    nc.vector.tensor_mul(h_new[:], tanh_cs[:], sig_o[:])
                yo = s * 4 * B_LOC
                nc.vector.tensor_mul(
                    ystore[:, yo : yo + 4 * B_LOC], tanh_c[:], sig_o[:]
                )

                if not norec:
                    prev_h = h_new
                    prev_c = c_new

            # flush this block's h outputs: y[cc, p, t0+s, b]
            # single contiguous DMA for the whole block so ystore slot
            # release costs one DMA-lane wait
            nc.gpsimd.dma_start(
                y_d[:, blk * TB : (blk + 1) * TB, :],
                ystore[:].rearrange("p (s cb) -> p s cb", s=TB),
            )

    nc.compile()
    return nc


def _get_program(T):
    if T not in _PROG_CACHE:
        _PROG_CACHE[T] = _build_program(T)
    return _PROG_CACHE[T]


def _prep_inputs(x, W, b, T):
    from concourse import mybir

    perm = np.concatenate(
        [
            np.arange(0, C_OUT),  # i
            np.arange(2 * C_OUT, 3 * C_OUT),  # f
            np.arange(C_OUT, 2 * C_OUT),  # j
            np.arange(3 * C_OUT, 4 * C_OUT),  # o
        ]
    )
    fp8 = mybir.dt.np(mybir.dt.float8e4)
    hsc = WH_SCALE if USE_DR else 1.0
    Wp = np.asarray(W, dtype=np.float32)[perm]
    wxT = np.ascontiguousarray(Wp[:, :C_IN].T * hsc).astype(ml_dtypes.bfloat16)
    whT = np.clip(
        np.ascontiguousarray(Wp[:, C_IN:].T) * WH_SCALE, -240.0, 240.0
    ).astype(fp8)
    if USE_DR == 2:
        whf = whT.astype(np.float32)  # [512, 2048]
        whP = np.empty((2, 128, NM * 256), np.float32)
        for p in range(2):
            for m in range(NM):
                A = whf[256 * p : 256 * p + 128, 128 * m : 128 * m + 128]
                Bc = whf[256 * p + 128 : 256 * p + 256, 128 * m : 128 * m + 128]
                blk = np.empty((128, 256), np.float32)
                blk[:, 0::2] = A[:, ::-1]
                blk[:, 1::2] = Bc[:, ::-1]
                whP[p][:, m * 256 : (m + 1) * 256] = blk
        whT = whP.astype(fp8)
    bmat = np.ascontiguousarray(
        np.asarray(b, dtype=np.float32)[perm].reshape(NM, 128).T * hsc
    )
    in_maps = []
    for kcore in range(N_CORES):
        xs = np.asarray(x[kcore * B_LOC : (kcore + 1) * B_LOC, :, :T], np.float32)
        xTc = np.ascontiguousarray(
            xs.transpose(1, 2, 0).reshape(C_IN, T * B_LOC)
        ).astype(ml_dtypes.bfloat16)
        in_maps.append({"xT": xTc, "wxT": wxT, "whT": whT, "bmat": bmat})
    return in_maps


def _assemble(results, T):
    out = np.empty((B, C_OUT, T), dtype=np.float32)
    for kcore in range(N_CORES):
        yk = np.asarray(results[kcore]["y"]).astype(np.float32)  # [128, T, 32]
        out[kcore * B_LOC : (kcore + 1) * B_LOC] = (
            yk.reshape(128, T, NKH, B_LOC).transpose(3, 2, 0, 1).reshape(
                B_LOC, C_OUT, T
            )
        )
    return out


def run(x, W, b, T=T_FULL, **spmd_kwargs):
    from concourse.bass_utils import run_bass_kernel_spmd

    nc = _get_program(T)
    in_maps = _prep_inputs(x, W, b, T)
    res = run_bass_kernel_spmd(nc, in_maps, core_ids=list(range(N_CORES)), **spmd_kwargs)
    return _assemble(res.results, T), res


def kernel(x, W, b):
    out, _ = run(x, W, b, T_FULL)
    return out
